# revision 17
# baseline (speedup 1.0000x reference)
"""Trainium2 Bass kernel for nn_Diagonal (grouped 3->1 banded linear).

Math (reference): out[b, o] = sum_{j=0..2} input[b, 3o+j] * weight[o, 3o+j] + bias[o]

Only the banded diagonal of `weight` matters: w_band[i] = weight[i//3, i].

Strategy: output-dim tensor parallelism across 8 NeuronCores (1250 outputs
each, padded to 10 o-tiles of 128; communication-free) with the whole
contraction on the PE as block-diagonal matmuls:

    outT[o, b] = sum_j diag(w_j[o-tile]) @ xT_j[o-tile, b]    (PSUM f32)

Host pre-transposes x into j-planes [3, O_pad, B] fp16, so every DMA row is
8KB-contiguous. The 128x128 diagonal stationaries are built on-chip
(identity (x) per-partition w column, VectorE tensor_scalar) from a tiny
[128, 3*NT] column tile. Because outputs live on partitions, the bias add
is free: ScalarE's PSUM->SBUF copy applies it as the per-partition
activation bias. fp16 in/out halves HBM traffic vs fp32 (the kernel is
memory-bound); rel-err ~7e-4 vs the fp32 reference.
"""

import os
import sys

import numpy as np

P = 128
B, I, O = 4096, 30000, 10000
N_CORES = 8
O_CORE = O // N_CORES          # 1250 outputs per core
NT = 10                        # o-tiles per core (1280 rows, 30 pad)
O_PAD = NT * P                 # 1280
NB = 8                         # batch tiles
BT = B // NB                   # 512 (= one PSUM bank of f32)

_CACHED = {}


def _build_nc():
    import concourse.bacc as bacc
    import concourse.mybir as mybir
    from concourse.tile import TileContext

    f32 = mybir.dt.float32
    f16 = mybir.dt.float16
    nc = bacc.Bacc(None, target_bir_lowering=False)

    # xT[j, o, b]: core's x slice, transposed, o padded to 1280
    x = nc.declare_dram_parameter("x", [3, O_PAD, B], f16, isOutput=False)
    # wcols[p, 3t+j] = w_j[o_base + 128 t + p]; bcols[p, t] = bias likewise
    wcols = nc.declare_dram_parameter("wcols", [P, 3 * NT], f32, isOutput=False)
    bcols = nc.declare_dram_parameter("bcols", [P, NT], f32, isOutput=False)
    ident = nc.declare_dram_parameter("ident", [P, P], f16, isOutput=False)
    y = nc.declare_dram_parameter("y", [O_PAD, B], f16, isOutput=True)

    with TileContext(nc) as tc:
        with (
            tc.tile_pool(name="colp", bufs=1) as colp,
            tc.tile_pool(name="dp", bufs=6) as dp,
            tc.tile_pool(name="xp", bufs=12) as xp,
            tc.tile_pool(name="psp", bufs=1, space="PSUM") as psp,
            tc.tile_pool(name="yp", bufs=2) as yp,
        ):
            id_t = colp.tile([P, P], f16, tag="ident")
            nc.sync.dma_start(out=id_t[:], in_=ident[:, :])
            wc_t = colp.tile([P, 3 * NT], f32, tag="wcols")
            nc.sync.dma_start(out=wc_t[:], in_=wcols[:, :])
            bc_t = colp.tile([P, NT], f32, tag="bcols")
            nc.sync.dma_start(out=bc_t[:], in_=bcols[:, :])

            for t in range(NT):
                # stationaries: D_j = diag(w_j[o-tile t])
                ds = []
                for j in range(3):
                    d = dp.tile([P, P], f16)
                    nc.vector.tensor_scalar_mul(
                        out=d[:], in0=id_t[:], scalar1=wc_t[:, 3 * t + j:3 * t + j + 1]
                    )
                    ds.append(d)
                # x planes for this o-tile: full batch row-strips (8KB rows)
                xs = []
                for j in range(3):
                    xt = xp.tile([P, B], f16)
                    nc.sync.dma_start(out=xt[:], in_=x[j, t * P:(t + 1) * P, :])
                    xs.append(xt)
                # 8 PSUM banks accumulate the 3 diagonal matmuls; j outer so
                # each stationary loads once per o-tile.
                pss = [
                    psp.tile([P, BT], f32, tag=f"ps{n}", name=f"ps{t}_{n}")
                    for n in range(NB)
                ]
                for j in range(3):
                    for n in range(NB):
                        nc.tensor.matmul(
                            pss[n][:],
                            ds[j][:],
                            xs[j][:, n * BT:(n + 1) * BT],
                            start=(j == 0), stop=(j == 2),
                        )
                # PSUM -> SBUF fp16 with fused per-partition bias add;
                # y flushed per half batch for finer DMA pipelining
                for h in range(2):
                    y_t = yp.tile([P, B // 2], f16)
                    for n in range(h * NB // 2, (h + 1) * NB // 2):
                        nc.scalar.activation(
                            out=y_t[:, (n - h * NB // 2) * BT:(n - h * NB // 2 + 1) * BT],
                            in_=pss[n][:],
                            func=mybir.ActivationFunctionType.Identity,
                            bias=bc_t[:, t:t + 1],
                        )
                    nc.sync.dma_start(
                        out=y[t * P:(t + 1) * P, h * (B // 2):(h + 1) * (B // 2)],
                        in_=y_t[:],
                    )
    nc.finalize()
    return nc


def _ensure_ntff_hook():
    """Register the axon NTFF profiling hook if the image's antenv lacks it."""
    import types

    name = "antenv.axon_hooks"
    mod = sys.modules.get(name)
    if mod is None:
        try:
            import antenv.axon_hooks as mod  # type: ignore
        except ImportError:
            mod = types.ModuleType(name)
            _state = {"hook": None}
            mod.set_axon_ntff_profile_hook = lambda h: _state.__setitem__("hook", h)
            mod.get_axon_ntff_profile_hook = lambda: _state["hook"]
            sys.modules[name] = mod
            import antenv
            antenv.axon_hooks = mod
    if mod.get_axon_ntff_profile_hook() is None:
        so = "/opt/axon/libaxon_pjrt.so"
        if os.path.exists(so):
            from trn_agent_boot.trn_boot import _ntff_profile_via_ctypes
            hook = _ntff_profile_via_ctypes(so)
            if hook is not None:
                mod.set_axon_ntff_profile_hook(hook)
    return mod.get_axon_ntff_profile_hook() is not None


def run_sharded(input, weight, bias, trace=False, tmpdir=None):
    """Run on 8 cores. Returns (full_output [B,O] f32, BassKernelResults)."""
    from concourse.bass_utils import run_bass_kernel_spmd

    x = np.asarray(input, dtype=np.float32)
    weight = np.asarray(weight, dtype=np.float32)
    bias = np.asarray(bias, dtype=np.float32)

    cols = np.arange(I)
    band = weight[cols // 3, cols].astype(np.float16)     # [I]
    planes = band.reshape(O, 3)                           # [O, 3]
    b16 = bias.astype(np.float16)

    # xT3[j, o, b] fp16 (one bulk transpose; per-core slices + pad below)
    x16 = x.astype(np.float16)
    xT3 = np.ascontiguousarray(x16.reshape(B, O, 3).transpose(2, 1, 0))

    ident = np.eye(P, dtype=np.float16)

    in_maps = []
    for c in range(N_CORES):
        o0 = c * O_CORE
        xc = np.zeros((3, O_PAD, B), dtype=np.float16)
        xc[:, :O_CORE, :] = xT3[:, o0:o0 + O_CORE, :]
        wc = np.zeros((P, 3 * NT), dtype=np.float32)
        bc = np.zeros((P, NT), dtype=np.float32)
        wpad = np.zeros((O_PAD, 3), dtype=np.float16)
        wpad[:O_CORE] = planes[o0:o0 + O_CORE]
        bpad = np.zeros(O_PAD, dtype=np.float16)
        bpad[:O_CORE] = b16[o0:o0 + O_CORE]
        for t in range(NT):
            for j in range(3):
                wc[:, 3 * t + j] = wpad[t * P:(t + 1) * P, j]
            bc[:, t] = bpad[t * P:(t + 1) * P]
        in_maps.append(
            {"x": xc, "wcols": wc, "bcols": bc, "ident": ident}
        )

    if "nc" not in _CACHED:
        _CACHED["nc"] = _build_nc()
    nc = _CACHED["nc"]

    kwargs = {}
    if trace:
        _ensure_ntff_hook()
        import concourse.bass_utils as bu
        bu.upload_artifacts = lambda d: d  # no fishfood/S3 in this container
        kwargs = {"trace": True, "tmpdir": tmpdir}

    res = run_bass_kernel_spmd(nc, in_maps, list(range(N_CORES)), **kwargs)
    out = np.empty((B, O), dtype=np.float32)
    for c in range(N_CORES):
        yT = res.results[c]["y"]                          # [O_PAD, B] f16
        out[:, c * O_CORE:(c + 1) * O_CORE] = yT[:O_CORE].T.astype(np.float32)
    return out, res


def kernel(input, weight, bias):
    out, _ = run_sharded(input, weight, bias, trace=False)
    return out


# revision 18
# speedup vs baseline: 1.0709x; 1.0709x over previous
"""Trainium2 Bass kernel for nn_Diagonal (grouped 3->1 banded linear).

Math (reference): out[b, o] = sum_{j=0..2} input[b, 3o+j] * weight[o, 3o+j] + bias[o]

Only the banded diagonal of `weight` matters: w_band[i] = weight[i//3, i].

Strategy: output-dim tensor parallelism across 8 NeuronCores (1250 outputs
each, padded to 10 o-tiles of 128; communication-free) with the whole
contraction on the PE as block-diagonal matmuls:

    outT[o, b] = sum_j diag(w_j[o-tile]) @ xT_j[o-tile, b]    (PSUM f32)

Host pre-transposes x into j-planes [3, O_pad, B] fp16, so every DMA row is
8KB-contiguous. The 128x128 diagonal stationaries are built on-chip
(identity (x) per-partition w column, VectorE tensor_scalar) from a tiny
[128, 3*NT] column tile. Because outputs live on partitions, the bias add
is free: ScalarE's PSUM->SBUF copy applies it as the per-partition
activation bias. fp16 in/out halves HBM traffic vs fp32 (the kernel is
memory-bound); rel-err ~7e-4 vs the fp32 reference.
"""

import os
import sys

import numpy as np

P = 128
B, I, O = 4096, 30000, 10000
N_CORES = 8
O_CORE = O // N_CORES          # 1250 outputs per core
NT = 10                        # o-tiles per core (1280 rows, 30 pad)
O_PAD = NT * P                 # 1280
NB = 8                         # batch tiles
BT = B // NB                   # 512 (= one PSUM bank of f32)

_CACHED = {}


def _build_nc():
    import concourse.bacc as bacc
    import concourse.mybir as mybir
    from concourse.tile import TileContext

    f32 = mybir.dt.float32
    f16 = mybir.dt.float16
    nc = bacc.Bacc(None, target_bir_lowering=False)

    # xT[j, o, b]: core's x slice, transposed, o padded to 1280
    x = nc.declare_dram_parameter("x", [3, O_PAD, B], f16, isOutput=False)
    # wcols[p, 3t+j] = w_j[o_base + 128 t + p]; bcols[p, t] = bias likewise
    wcols = nc.declare_dram_parameter("wcols", [P, 3 * NT], f32, isOutput=False)
    bcols = nc.declare_dram_parameter("bcols", [P, NT], f32, isOutput=False)
    ident = nc.declare_dram_parameter("ident", [P, P], f16, isOutput=False)
    y = nc.declare_dram_parameter("y", [O_PAD, B], f16, isOutput=True)

    with TileContext(nc) as tc:
        with (
            tc.tile_pool(name="colp", bufs=1) as colp,
            tc.tile_pool(name="dp", bufs=6) as dp,
            tc.tile_pool(name="xp", bufs=12) as xp,
            tc.tile_pool(name="psp", bufs=1, space="PSUM") as psp,
            tc.tile_pool(name="yp", bufs=2) as yp,
        ):
            id_t = colp.tile([P, P], f16, tag="ident")
            nc.sync.dma_start(out=id_t[:], in_=ident[:, :])
            wc_t = colp.tile([P, 3 * NT], f32, tag="wcols")
            nc.sync.dma_start(out=wc_t[:], in_=wcols[:, :])
            bc_t = colp.tile([P, NT], f32, tag="bcols")
            nc.sync.dma_start(out=bc_t[:], in_=bcols[:, :])

            for t in range(NT):
                # stationaries: D_j = diag(w_j[o-tile t])
                ds = []
                for j in range(3):
                    d = dp.tile([P, P], f16)
                    nc.vector.tensor_scalar_mul(
                        out=d[:], in0=id_t[:], scalar1=wc_t[:, 3 * t + j:3 * t + j + 1]
                    )
                    ds.append(d)
                # x planes for this o-tile: full batch row-strips (8KB rows)
                xs = []
                for j in range(3):
                    xt = xp.tile([P, B], f16)
                    nc.sync.dma_start(out=xt[:], in_=x[j, t * P:(t + 1) * P, :])
                    xs.append(xt)
                # 8 PSUM banks accumulate the 3 diagonal matmuls; j outer so
                # each stationary loads once per o-tile.
                pss = [
                    psp.tile([P, BT], f32, tag=f"ps{n}", name=f"ps{t}_{n}")
                    for n in range(NB)
                ]
                for j in range(3):
                    for n in range(NB):
                        nc.tensor.matmul(
                            pss[n][:],
                            ds[j][:],
                            xs[j][:, n * BT:(n + 1) * BT],
                            start=(j == 0), stop=(j == 2),
                        )
                # PSUM -> SBUF fp16 with fused per-partition bias add;
                # y flushed per half batch for finer DMA pipelining
                for h in range(2):
                    y_t = yp.tile([P, B // 2], f16)
                    for n in range(h * NB // 2, (h + 1) * NB // 2):
                        nc.scalar.activation(
                            out=y_t[:, (n - h * NB // 2) * BT:(n - h * NB // 2 + 1) * BT],
                            in_=pss[n][:],
                            func=mybir.ActivationFunctionType.Identity,
                            bias=bc_t[:, t:t + 1],
                        )
                    # scalar (not sync) issues y: the sync queue must never
                    # wait on ACT, or x prefetch serializes behind it
                    nc.scalar.dma_start(
                        out=y[t * P:(t + 1) * P, h * (B // 2):(h + 1) * (B // 2)],
                        in_=y_t[:],
                    )
    nc.finalize()
    return nc


def _ensure_ntff_hook():
    """Register the axon NTFF profiling hook if the image's antenv lacks it."""
    import types

    name = "antenv.axon_hooks"
    mod = sys.modules.get(name)
    if mod is None:
        try:
            import antenv.axon_hooks as mod  # type: ignore
        except ImportError:
            mod = types.ModuleType(name)
            _state = {"hook": None}
            mod.set_axon_ntff_profile_hook = lambda h: _state.__setitem__("hook", h)
            mod.get_axon_ntff_profile_hook = lambda: _state["hook"]
            sys.modules[name] = mod
            import antenv
            antenv.axon_hooks = mod
    if mod.get_axon_ntff_profile_hook() is None:
        so = "/opt/axon/libaxon_pjrt.so"
        if os.path.exists(so):
            from trn_agent_boot.trn_boot import _ntff_profile_via_ctypes
            hook = _ntff_profile_via_ctypes(so)
            if hook is not None:
                mod.set_axon_ntff_profile_hook(hook)
    return mod.get_axon_ntff_profile_hook() is not None


def run_sharded(input, weight, bias, trace=False, tmpdir=None):
    """Run on 8 cores. Returns (full_output [B,O] f32, BassKernelResults)."""
    from concourse.bass_utils import run_bass_kernel_spmd

    x = np.asarray(input, dtype=np.float32)
    weight = np.asarray(weight, dtype=np.float32)
    bias = np.asarray(bias, dtype=np.float32)

    cols = np.arange(I)
    band = weight[cols // 3, cols].astype(np.float16)     # [I]
    planes = band.reshape(O, 3)                           # [O, 3]
    b16 = bias.astype(np.float16)

    # xT3[j, o, b] fp16 (one bulk transpose; per-core slices + pad below)
    x16 = x.astype(np.float16)
    xT3 = np.ascontiguousarray(x16.reshape(B, O, 3).transpose(2, 1, 0))

    ident = np.eye(P, dtype=np.float16)

    in_maps = []
    for c in range(N_CORES):
        o0 = c * O_CORE
        xc = np.zeros((3, O_PAD, B), dtype=np.float16)
        xc[:, :O_CORE, :] = xT3[:, o0:o0 + O_CORE, :]
        wc = np.zeros((P, 3 * NT), dtype=np.float32)
        bc = np.zeros((P, NT), dtype=np.float32)
        wpad = np.zeros((O_PAD, 3), dtype=np.float16)
        wpad[:O_CORE] = planes[o0:o0 + O_CORE]
        bpad = np.zeros(O_PAD, dtype=np.float16)
        bpad[:O_CORE] = b16[o0:o0 + O_CORE]
        for t in range(NT):
            for j in range(3):
                wc[:, 3 * t + j] = wpad[t * P:(t + 1) * P, j]
            bc[:, t] = bpad[t * P:(t + 1) * P]
        in_maps.append(
            {"x": xc, "wcols": wc, "bcols": bc, "ident": ident}
        )

    if "nc" not in _CACHED:
        _CACHED["nc"] = _build_nc()
    nc = _CACHED["nc"]

    kwargs = {}
    if trace:
        _ensure_ntff_hook()
        import concourse.bass_utils as bu
        bu.upload_artifacts = lambda d: d  # no fishfood/S3 in this container
        kwargs = {"trace": True, "tmpdir": tmpdir}

    res = run_bass_kernel_spmd(nc, in_maps, list(range(N_CORES)), **kwargs)
    out = np.empty((B, O), dtype=np.float32)
    for c in range(N_CORES):
        yT = res.results[c]["y"]                          # [O_PAD, B] f16
        out[:, c * O_CORE:(c + 1) * O_CORE] = yT[:O_CORE].T.astype(np.float32)
    return out, res


def kernel(input, weight, bias):
    out, _ = run_sharded(input, weight, bias, trace=False)
    return out


# revision 20
# speedup vs baseline: 1.3666x; 1.2762x over previous
"""Trainium2 Bass kernel for nn_Diagonal (grouped 3->1 banded linear).

Math (reference): out[b, o] = sum_{j=0..2} input[b, 3o+j] * weight[o, 3o+j] + bias[o]

Only the banded diagonal of `weight` matters: w_band[i] = weight[i//3, i].

Strategy: output-dim tensor parallelism across 8 NeuronCores (1250 outputs
each, padded to 10 o-tiles of 128; communication-free) with the whole
contraction on the PE as block-diagonal matmuls:

    outT[o, b] = sum_j diag(w_j[o-tile]) @ xT_j[o-tile, b]    (PSUM f32)

Host pre-transposes x into j-planes [3, O_pad, B] fp16, so every DMA row is
8KB-contiguous. The 128x128 diagonal stationaries are built on-chip
(identity (x) per-partition w column, VectorE tensor_scalar) from a tiny
[128, 3*NT] column tile. Because outputs live on partitions, the bias add
is free: ScalarE's PSUM->SBUF copy applies it as the per-partition
activation bias. fp16 in/out halves HBM traffic vs fp32 (the kernel is
memory-bound); rel-err ~7e-4 vs the fp32 reference.
"""

import os
import sys

import numpy as np

P = 128
B, I, O = 4096, 30000, 10000
N_CORES = 8
O_CORE = O // N_CORES          # 1250 outputs per core
NT = 10                        # o-tiles per core (1280 rows, 30 pad)
O_PAD = NT * P                 # 1280
NB = 8                         # batch tiles
BT = B // NB                   # 512 (= one PSUM bank of f32)

_CACHED = {}


def _build_nc():
    import concourse.bacc as bacc
    import concourse.mybir as mybir
    from concourse.tile import TileContext

    f32 = mybir.dt.float32
    f16 = mybir.dt.float16
    nc = bacc.Bacc(None, target_bir_lowering=False)

    # xT[j, o, b]: core's x slice, transposed, o padded to 1280
    x = nc.declare_dram_parameter("x", [3, O_PAD, B], f16, isOutput=False)
    # wcols[p, 3t+j] = w_j[o_base + 128 t + p]; bcols[p, t] = bias likewise
    wcols = nc.declare_dram_parameter("wcols", [P, 3 * NT], f32, isOutput=False)
    bcols = nc.declare_dram_parameter("bcols", [P, NT], f32, isOutput=False)
    ident = nc.declare_dram_parameter("ident", [P, P], f16, isOutput=False)
    y = nc.declare_dram_parameter("y", [O_PAD, B], f16, isOutput=True)

    with TileContext(nc) as tc:
        with (
            tc.tile_pool(name="colp", bufs=1) as colp,
            tc.tile_pool(name="dp", bufs=2) as dp,
            tc.tile_pool(name="xp", bufs=4) as xp,
            tc.tile_pool(name="psp", bufs=1, space="PSUM") as psp,
            tc.tile_pool(name="yp", bufs=2) as yp,
        ):
            id_t = colp.tile([P, P], f16, tag="ident")
            nc.sync.dma_start(out=id_t[:], in_=ident[:, :])
            wc_t = colp.tile([P, 3 * NT], f32, tag="wcols")
            nc.sync.dma_start(out=wc_t[:], in_=wcols[:, :])
            bc_t = colp.tile([P, NT], f32, tag="bcols")
            nc.sync.dma_start(out=bc_t[:], in_=bcols[:, :])

            # Software-pipelined emission, LOOKAHEAD tiles deep: the sync
            # engine issues x DMAs for tiles t..t+LOOKAHEAD before it reaches
            # y(t)'s ACT-dependent wait, so x prefetch never starves and y
            # transfers stay interleaved with x in the same queue.
            LOOKAHEAD = 2
            xq = {}

            def emit_x(t):
                xs = []
                for j in range(3):
                    xt = xp.tile([P, B], f16, tag=f"x{j}", name=f"x{t}_{j}")
                    nc.sync.dma_start(out=xt[:], in_=x[j, t * P:(t + 1) * P, :])
                    xs.append(xt)
                xq[t] = xs

            def emit_compute(t):
                xs = xq.pop(t)
                ds = []
                for j in range(3):
                    d = dp.tile([P, P], f16, tag=f"d{j}", name=f"d{t}_{j}")
                    nc.vector.tensor_scalar_mul(
                        out=d[:], in0=id_t[:],
                        scalar1=wc_t[:, 3 * t + j:3 * t + j + 1],
                    )
                    ds.append(d)
                pss = [
                    psp.tile([P, BT], f32, tag=f"ps{n}", name=f"ps{t}_{n}")
                    for n in range(NB)
                ]
                for j in range(3):
                    for n in range(NB):
                        nc.tensor.matmul(
                            pss[n][:],
                            ds[j][:],
                            xs[j][:, n * BT:(n + 1) * BT],
                            start=(j == 0), stop=(j == 2),
                        )
                # PSUM -> SBUF fp16 with fused per-partition bias add
                for h in range(2):
                    y_t = yp.tile([P, B // 2], f16, tag=f"y{h}", name=f"y{t}_{h}")
                    for n in range(h * NB // 2, (h + 1) * NB // 2):
                        nc.scalar.activation(
                            out=y_t[:, (n - h * NB // 2) * BT:(n - h * NB // 2 + 1) * BT],
                            in_=pss[n][:],
                            func=mybir.ActivationFunctionType.Identity,
                            bias=bc_t[:, t:t + 1],
                        )
                    nc.sync.dma_start(
                        out=y[t * P:(t + 1) * P, h * (B // 2):(h + 1) * (B // 2)],
                        in_=y_t[:],
                    )

            for t in range(NT + LOOKAHEAD):
                if t < NT:
                    emit_x(t)
                if t >= LOOKAHEAD:
                    emit_compute(t - LOOKAHEAD)
    nc.finalize()
    return nc


def _ensure_ntff_hook():
    """Register the axon NTFF profiling hook if the image's antenv lacks it."""
    import types

    name = "antenv.axon_hooks"
    mod = sys.modules.get(name)
    if mod is None:
        try:
            import antenv.axon_hooks as mod  # type: ignore
        except ImportError:
            mod = types.ModuleType(name)
            _state = {"hook": None}
            mod.set_axon_ntff_profile_hook = lambda h: _state.__setitem__("hook", h)
            mod.get_axon_ntff_profile_hook = lambda: _state["hook"]
            sys.modules[name] = mod
            import antenv
            antenv.axon_hooks = mod
    if mod.get_axon_ntff_profile_hook() is None:
        so = "/opt/axon/libaxon_pjrt.so"
        if os.path.exists(so):
            from trn_agent_boot.trn_boot import _ntff_profile_via_ctypes
            hook = _ntff_profile_via_ctypes(so)
            if hook is not None:
                mod.set_axon_ntff_profile_hook(hook)
    return mod.get_axon_ntff_profile_hook() is not None


def run_sharded(input, weight, bias, trace=False, tmpdir=None):
    """Run on 8 cores. Returns (full_output [B,O] f32, BassKernelResults)."""
    from concourse.bass_utils import run_bass_kernel_spmd

    x = np.asarray(input, dtype=np.float32)
    weight = np.asarray(weight, dtype=np.float32)
    bias = np.asarray(bias, dtype=np.float32)

    cols = np.arange(I)
    band = weight[cols // 3, cols].astype(np.float16)     # [I]
    planes = band.reshape(O, 3)                           # [O, 3]
    b16 = bias.astype(np.float16)

    # xT3[j, o, b] fp16 (one bulk transpose; per-core slices + pad below)
    x16 = x.astype(np.float16)
    xT3 = np.ascontiguousarray(x16.reshape(B, O, 3).transpose(2, 1, 0))

    ident = np.eye(P, dtype=np.float16)

    in_maps = []
    for c in range(N_CORES):
        o0 = c * O_CORE
        xc = np.zeros((3, O_PAD, B), dtype=np.float16)
        xc[:, :O_CORE, :] = xT3[:, o0:o0 + O_CORE, :]
        wc = np.zeros((P, 3 * NT), dtype=np.float32)
        bc = np.zeros((P, NT), dtype=np.float32)
        wpad = np.zeros((O_PAD, 3), dtype=np.float16)
        wpad[:O_CORE] = planes[o0:o0 + O_CORE]
        bpad = np.zeros(O_PAD, dtype=np.float16)
        bpad[:O_CORE] = b16[o0:o0 + O_CORE]
        for t in range(NT):
            for j in range(3):
                wc[:, 3 * t + j] = wpad[t * P:(t + 1) * P, j]
            bc[:, t] = bpad[t * P:(t + 1) * P]
        in_maps.append(
            {"x": xc, "wcols": wc, "bcols": bc, "ident": ident}
        )

    if "nc" not in _CACHED:
        _CACHED["nc"] = _build_nc()
    nc = _CACHED["nc"]

    kwargs = {}
    if trace:
        _ensure_ntff_hook()
        import concourse.bass_utils as bu
        bu.upload_artifacts = lambda d: d  # no fishfood/S3 in this container
        kwargs = {"trace": True, "tmpdir": tmpdir}

    res = run_bass_kernel_spmd(nc, in_maps, list(range(N_CORES)), **kwargs)
    out = np.empty((B, O), dtype=np.float32)
    for c in range(N_CORES):
        yT = res.results[c]["y"]                          # [O_PAD, B] f16
        out[:, c * O_CORE:(c + 1) * O_CORE] = yT[:O_CORE].T.astype(np.float32)
    return out, res


def kernel(input, weight, bias):
    out, _ = run_sharded(input, weight, bias, trace=False)
    return out
